# revision 22
# baseline (speedup 1.0000x reference)
"""Trainium2 Bass kernel for nn_AttentionModule (Bahdanau-style attention).

Reference computation (S=512, B=64, H=1024, F=2H):
    cat    = concat([hidden bcast to (S,B,H), encoder_states], -1)      [S,B,2H]
    scores = tanh(cat @ W_attn.T + b_attn) @ W_attn2.T + b_attn2        [S,B,1]
    attn   = softmax(scores[..., 0].T, axis=-1)                         [B,S]
    applied= einsum("bs,sbh->bh", attn, encoder_states)                 [B,H]
    out    = tanh(concat([decoder_out, applied], -1) @ W_comb.T + b_comb)

Sharding: data-parallel over B across 8 cores (8 batch rows per core).

v4 structure:
  - Main matmul T^T[f,s] = sum_h W2T[h,f]*encT[h,s] in fp8e4m3 DoubleRow
    (256 contraction rows per instruction).  W2 host-scaled by 256; the
    tanh undoes it via its fused scale=1/256.  W2T is laid out per-f-tile
    so the first DR group only waits on a 128KB DMA.
  - The first four DR groups of batch row 0 are emitted BEFORE the
    preamble so the PE has work while W1 streams in.
  - hid@W1.T preamble: W1 resident [128, KH*F], 16x256KB DMAs issued on
    the (idle) scalar queue in consumption order; 32 bf16 matmuls + PE
    transposes.  Scores bias hidb folded into tanh as per-partition bias.
  - Scores matmul (attn2) in bf16 with W_attn2 replicated to 128 columns:
    the psum scores tile [128, S] carries the row broadcast for free
    (fp8 tanh outputs measurably hurt the attention weights, so attn2
    stays bf16).
  - Softmax skips max-subtraction (scores are provably tiny); Exp fuses
    the sum via accum_out.
  - applied^T: bf16 mult+reduce per h-chunk on VectorE over a bf16
    encoder copy (gpsimd is too slow per-op to help).
  - enc fp8/bf16 copies streamed per batch row (prefetch one ahead).
  - Final combine bf16; decoder half emitted before the last batch row
    so the PE tail only waits on the last row's applied.
"""

import numpy as np

S, B, H = 512, 64, 1024
F = 2 * H
NCORES = 8
BL = B // NCORES          # 8 batch rows per core
KH = H // 128             # 8 contraction chunks over H
KF = F // 128             # 16 feature tiles
W2SCALE = 256.0           # host pre-scale on W2 for fp8 range

_CACHE = {}


def _build(num_devices=NCORES):
    from contextlib import ExitStack

    import concourse.tile as tile
    from concourse import bacc, mybir
    from concourse.masks import make_identity

    f32 = mybir.dt.float32
    bf16 = mybir.dt.bfloat16
    f8 = mybir.dt.float8e4
    AF = mybir.ActivationFunctionType
    ALU = mybir.AluOpType
    AX = mybir.AxisListType
    DR = mybir.MatmulPerfMode.DoubleRow

    nc = bacc.Bacc("TRN2", target_bir_lowering=False, debug=False,
                   num_devices=num_devices)

    # enc free layout per partition: [b, kc, s]; h = kc*128 + p
    enc8_d = nc.dram_tensor("enc8", [128, BL * KH * S], f8,
                            kind="ExternalInput").ap()
    encb_d = nc.dram_tensor("encb", [128, BL * KH * S], bf16,
                            kind="ExternalInput").ap()
    # w2t8 free layout: [ft, kc, f]; wat1 free layout: [kc, f]
    w2t8_d = nc.dram_tensor("w2t8", [128, KF * KH * 128], f8,
                            kind="ExternalInput").ap()
    wat1_d = nc.dram_tensor("wat1", [128, KH * F], bf16,
                            kind="ExternalInput").ap()
    wct = nc.dram_tensor("wct", [F, H], bf16, kind="ExternalInput").ap()
    hidT = nc.dram_tensor("hidT", [H, BL], f32, kind="ExternalInput").ap()
    decT = nc.dram_tensor("decT", [H, BL], f32, kind="ExternalInput").ap()
    w2rep = nc.dram_tensor("w2rep", [F, 128], f32, kind="ExternalInput").ap()
    b_attn_d = nc.dram_tensor("b_attn", [1, F], f32, kind="ExternalInput").ap()
    b_comb_d = nc.dram_tensor("b_comb", [1, H], f32, kind="ExternalInput").ap()
    out_d = nc.dram_tensor("out", [BL, H], f32, kind="ExternalOutput").ap()
    appT_d = nc.dram_tensor("appliedT", [H, BL], f32,
                            kind="ExternalOutput").ap()

    with tile.TileContext(nc) as tc:
        with ExitStack() as ctx:
            consts = ctx.enter_context(tc.tile_pool(name="consts", bufs=1))
            enc8_pool = ctx.enter_context(tc.tile_pool(name="enc8p", bufs=2))
            encb_pool = ctx.enter_context(tc.tile_pool(name="encbp", bufs=2))
            tanh_pool = ctx.enter_context(tc.tile_pool(name="tanh", bufs=18))
            attn_pool = ctx.enter_context(tc.tile_pool(name="attn", bufs=2))
            scr_pool = ctx.enter_context(tc.tile_pool(name="scr", bufs=2))
            small_pool = ctx.enter_context(tc.tile_pool(name="small", bufs=4))
            wct_pool = ctx.enter_context(tc.tile_pool(name="wct", bufs=4))
            wcta_pool = ctx.enter_context(tc.tile_pool(name="wcta", bufs=8))
            psT_pool = ctx.enter_context(
                tc.tile_pool(name="psT", bufs=4, space="PSUM"))
            psSc_pool = ctx.enter_context(
                tc.tile_pool(name="psSc", bufs=2, space="PSUM"))
            psPre_pool = ctx.enter_context(
                tc.tile_pool(name="psPre", bufs=2, space="PSUM"))

            # ---- W1 on the scalar queue (idle early), consumption order ----
            w1_sb = consts.tile([128, KH * F], bf16)
            for j in range(2):
                for kc in range(KH):
                    nc.scalar.dma_start(
                        w1_sb[:, kc * F + j * 1024: kc * F + (j + 1) * 1024],
                        wat1_d[:, kc * F + j * 1024: kc * F + (j + 1) * 1024])

            # ---- sync queue: W2 head chunk, enc b0, then the rest ----
            w2t8_sb = consts.tile([128, KF * KH * 128], f8)
            CW = KH * 128

            def w2_load(ft):
                nc.sync.dma_start(w2t8_sb[:, ft * CW:(ft + 1) * CW],
                                  w2t8_d[:, ft * CW:(ft + 1) * CW])

            def enc8_load(b):
                t = enc8_pool.tile([128, KH * S], f8, tag="enc8", name="enc8")
                half = KH * S // 2
                for i in range(2):
                    nc.sync.dma_start(
                        t[:, i * half:(i + 1) * half],
                        enc8_d[:, b * KH * S + i * half:
                               b * KH * S + (i + 1) * half])
                return t

            def encb_load(b):
                t = encb_pool.tile([128, KH * S], bf16, tag="encb",
                                   name="encb")
                q = KH * S // 4
                for i in range(4):
                    nc.sync.dma_start(
                        t[:, i * q:(i + 1) * q],
                        encb_d[:, b * KH * S + i * q:
                               b * KH * S + (i + 1) * q])
                return t

            w2_load(0)
            enc8_tiles = {0: enc8_load(0)}
            for ft in range(1, 4):
                w2_load(ft)

            # ---- small constants (shipped fp32, cast on device) ----
            identity = consts.tile([128, 128], f32)
            make_identity(nc, identity[:])
            ones_bf = consts.tile([1, BL], bf16)
            nc.vector.memset(ones_bf[:], 1.0)
            hidT_32 = consts.tile([128, KH * BL], f32)
            for kc in range(KH):
                nc.sync.dma_start(hidT_32[:, kc * BL:(kc + 1) * BL],
                                  hidT[kc * 128:(kc + 1) * 128, :])
            hidT_sb = consts.tile([128, KH * BL], bf16)
            nc.vector.tensor_copy(hidT_sb[:], hidT_32[:])
            b_attn_32 = consts.tile([1, F], f32)
            nc.sync.dma_start(b_attn_32[:], b_attn_d[:])
            b_attn_sb = consts.tile([1, F], bf16)
            nc.vector.tensor_copy(b_attn_sb[:], b_attn_32[:])
            w2rep_32 = consts.tile([128, KF * 128], f32)
            for i in range(4):
                nc.sync.dma_start(
                    w2rep_32[:, i * 512:(i + 1) * 512].rearrange(
                        "p (a c) -> p a c", a=4),
                    w2rep[i * 512:(i + 1) * 512, :].rearrange(
                        "(a p) c -> p a c", p=128))
            w2rep_sb = consts.tile([128, KF * 128], bf16)
            nc.vector.tensor_copy(w2rep_sb[:], w2rep_32[:])

            for ft in range(4, KF):
                w2_load(ft)

            decT_32 = consts.tile([128, KH * BL], f32)
            for kc in range(KH):
                nc.sync.dma_start(decT_32[:, kc * BL:(kc + 1) * BL],
                                  decT[kc * 128:(kc + 1) * 128, :])
            decT_sb = consts.tile([128, KH * BL], bf16)
            nc.vector.tensor_copy(decT_sb[:], decT_32[:])
            b_comb_32 = consts.tile([1, H], f32)
            nc.sync.dma_start(b_comb_32[:], b_comb_d[:])
            b_comb_sb = consts.tile([1, H], bf16)
            nc.vector.tensor_copy(b_comb_sb[:], b_comb_32[:])
            encb_tiles = {0: encb_load(0)}

            hidbT_sb = consts.tile([128, KF * BL], f32)
            appT_sb = consts.tile([128, KH * BL], f32)
            appT_bf = consts.tile([128, KH * BL], bf16)
            w2t8_v = w2t8_sb.rearrange("p (t k f) -> p t k f", t=KF, k=KH)

            # ---- prologue: first 4 DR groups of b0 keep the PE busy while
            # W1 streams in for the preamble ----
            def dr_group(et8_v, ft):
                pT = psT_pool.tile([128, S], f32, tag="pT", name="pT")
                for kc2 in range(KH // 2):
                    nc.tensor.matmul(
                        pT[:],
                        w2t8_v[:, ft, 2 * kc2:2 * kc2 + 2, :],
                        et8_v[:, 2 * kc2:2 * kc2 + 2, :],
                        start=(kc2 == 0), stop=(kc2 == KH // 2 - 1),
                        perf_mode=DR)
                return pT

            et8_b0 = enc8_tiles.pop(0)
            et8_b0_v = et8_b0.rearrange("p (k s) -> p k s", k=KH)
            pT_pending = [dr_group(et8_b0_v, ft) for ft in range(4)]

            # ---- hid_part preamble: hidb[b, f] = hidden @ W1.T + b_attn ----
            hidb_row = consts.tile([BL, F], f32)
            for fc in range(F // 512):
                ph = psPre_pool.tile([BL, 512], f32, tag="pre", name=f"ph{fc}")
                for kc in range(KH):
                    nc.tensor.matmul(
                        ph[:], hidT_sb[:, kc * BL:(kc + 1) * BL],
                        w1_sb[:, kc * F + fc * 512: kc * F + (fc + 1) * 512],
                        start=(kc == 0), stop=False)
                nc.tensor.matmul(
                    ph[:], ones_bf[:], b_attn_sb[:, fc * 512:(fc + 1) * 512],
                    start=False, stop=True)
                nc.vector.tensor_copy(hidb_row[:, fc * 512:(fc + 1) * 512],
                                      ph[:])
            for ft in range(KF):
                ptp = psPre_pool.tile([128, BL], f32, tag="pre", name="ptp")
                nc.tensor.transpose(ptp[:],
                                    hidb_row[:, ft * 128:(ft + 1) * 128],
                                    identity[:BL, :BL])
                nc.vector.tensor_copy(hidbT_sb[:, ft * BL:(ft + 1) * BL],
                                      ptp[:])

            # ---- main loop over local batch rows ----
            inv = 1.0 / W2SCALE
            pouts = [None, None]
            for b in range(BL):
                if b + 1 < BL:
                    enc8_tiles[b + 1] = enc8_load(b + 1)
                    encb_tiles[b + 1] = encb_load(b + 1)
                if b > 0:
                    et8 = enc8_tiles.pop(b)
                    et8_v = et8.rearrange("p (k s) -> p k s", k=KH)
                else:
                    et8_v = et8_b0_v

                psc = psSc_pool.tile([128, S], f32, tag="psc", name="psc")
                th = [None] * KF

                def attn2(ft):
                    nc.tensor.matmul(
                        psc[:],
                        w2rep_sb[:, ft * 128:(ft + 1) * 128],
                        th[ft][:],
                        start=(ft == 0), stop=(ft == KF - 1))

                for ft in range(KF):
                    if b == 0 and ft < 4:
                        pT = pT_pending[ft]
                    else:
                        pT = dr_group(et8_v, ft)
                    t = tanh_pool.tile([128, S], bf16, tag="tanh", name="tanh")
                    nc.scalar.activation(
                        t[:], pT[:], AF.Tanh,
                        bias=hidbT_sb[:, ft * BL + b: ft * BL + b + 1],
                        scale=inv)
                    th[ft] = t
                    if ft >= 2:
                        attn2(ft - 2)
                attn2(KF - 2)
                attn2(KF - 1)

                # softmax over s (no max subtraction: |scores| <~ 2)
                attn = attn_pool.tile([128, S], bf16, tag="attn", name="attn")
                sumexp = small_pool.tile([128, 1], f32, tag="sumexp",
                                         name="sumexp")
                nc.scalar.activation(attn[:], psc[:], AF.Exp,
                                     bias=0.0, scale=1.0,
                                     accum_out=sumexp[:])
                recip = small_pool.tile([128, 1], f32, tag="recip",
                                        name="recip")
                nc.vector.reciprocal(recip[:], sumexp[:])
                nc.vector.tensor_scalar_mul(attn[:], attn[:], recip[:])

                # applied^T[h, b]: bf16 mult+reduce on VectorE.  For the last
                # row, each finished h-chunk immediately feeds its final-
                # combine matmul so the PE tail never waits on the full row.
                etb = encb_tiles.pop(b)
                etb_v = etb.rearrange("p (k s) -> p k s", k=KH)
                for kc in range(KH):
                    scr = scr_pool.tile([128, S], bf16, tag="scr", name="scr")
                    nc.vector.tensor_tensor(out=scr[:], in0=etb_v[:, kc, :],
                                            in1=attn[:], op=ALU.mult)
                    nc.vector.reduce_sum(
                        appT_sb[:, kc * BL + b: kc * BL + b + 1],
                        scr[:], axis=AX.X)
                    if b == BL - 1:
                        nc.vector.tensor_copy(
                            appT_bf[:, kc * BL:(kc + 1) * BL],
                            appT_sb[:, kc * BL:(kc + 1) * BL])
                        w = wct_tiles[kc]
                        for fc in range(2):
                            nc.tensor.matmul(
                                pouts[fc][:],
                                appT_bf[:, kc * BL:(kc + 1) * BL],
                                w[:, fc * 512:(fc + 1) * 512],
                                start=False, stop=False)

                # decoder half of the final combine before the last row
                if b == BL - 2:
                    for i in range(2):
                        pouts[i] = psPre_pool.tile([BL, 512], f32, tag="pre",
                                                   name=f"po{i}")
                    for kc in range(KH):
                        w = wct_pool.tile([128, H], bf16, tag="wct",
                                          name="wctt")
                        nc.sync.dma_start(w[:], wct[kc * 128:(kc + 1) * 128, :])
                        for fc in range(2):
                            nc.tensor.matmul(
                                pouts[fc][:],
                                decT_sb[:, kc * BL:(kc + 1) * BL],
                                w[:, fc * 512:(fc + 1) * 512],
                                start=(kc == 0), stop=False)
                    # applied-half weights, loaded ahead of the last row
                    wct_tiles = []
                    for kc in range(KH):
                        w = wcta_pool.tile([128, H], bf16, tag="wcta",
                                           name="wcta")
                        nc.sync.dma_start(w[:], wct[(KH + kc) * 128:
                                                    (KH + kc + 1) * 128, :])
                        wct_tiles.append(w)

            # ---- final combine: bias, tanh ----
            for fc in range(2):
                nc.tensor.matmul(
                    pouts[fc][:], ones_bf[:],
                    b_comb_sb[:, fc * 512:(fc + 1) * 512],
                    start=False, stop=True)

            out_sb = consts.tile([BL, H], f32)
            for fc in range(2):
                nc.scalar.activation(out_sb[:, fc * 512:(fc + 1) * 512],
                                     pouts[fc][:], AF.Tanh)
            nc.sync.dma_start(out_d[:], out_sb[:])
            for kc in range(KH):
                nc.sync.dma_start(appT_d[kc * 128:(kc + 1) * 128, :],
                                  appT_sb[:, kc * BL:(kc + 1) * BL])

    nc.compile()
    return nc


def _get_nc():
    if "nc" not in _CACHE:
        _CACHE["nc"] = _build()
    return _CACHE["nc"]


def make_in_maps(inputs):
    import ml_dtypes
    bf = ml_dtypes.bfloat16
    f8 = ml_dtypes.float8_e4m3

    inp = {k: np.asarray(v, dtype=np.float32) for k, v in inputs.items()}
    hidden = inp["hidden"]
    decoder_out = inp["decoder_out"]
    encoder_states = inp["encoder_states"]
    W_attn = inp["W_attn"]
    b_attn = inp["b_attn"]
    W_attn2 = inp["W_attn2"]
    W_comb = inp["W_comb"]
    b_comb = inp["b_comb"]
    # b_attn2 shifts every score equally -> softmax-invariant, unused.

    wat = np.ascontiguousarray(W_attn.T)                    # [F, F] fp32
    wat1 = np.ascontiguousarray(
        wat[:H].reshape(KH, 128, F).transpose(1, 0, 2)
        .reshape(128, KH * F)).astype(bf)
    w2t8 = np.ascontiguousarray(
        (wat[H:] * W2SCALE).reshape(KH, 128, KF, 128)
        .transpose(1, 2, 0, 3).reshape(128, KF * KH * 128)).astype(f8)
    wct = np.ascontiguousarray(W_comb.T).astype(bf)
    w2rep = np.ascontiguousarray(np.repeat(W_attn2.reshape(F, 1), 128, axis=1))
    b_attn_2d = np.ascontiguousarray(b_attn.reshape(1, F))
    b_comb_2d = np.ascontiguousarray(b_comb.reshape(1, H))

    in_maps = []
    for c in range(NCORES):
        sl = slice(c * BL, (c + 1) * BL)
        # [S, BL, H] -> [BL, H, S] -> [BL, KH, 128, S] -> [128, BL, KH, S]
        enc = np.ascontiguousarray(
            encoder_states[:, sl, :].transpose(1, 2, 0)
            .reshape(BL, KH, 128, S).transpose(2, 0, 1, 3)
            .reshape(128, BL * KH * S))
        in_maps.append({
            "enc8": enc.astype(f8),
            "encb": enc.astype(bf),
            "w2t8": w2t8,
            "wat1": wat1,
            "wct": wct,
            "hidT": np.ascontiguousarray(hidden[sl].T),
            "decT": np.ascontiguousarray(decoder_out[sl].T),
            "w2rep": w2rep,
            "b_attn": b_attn_2d,
            "b_comb": b_comb_2d,
        })
    return in_maps


def kernel(**inputs):
    from concourse.bass_utils import run_bass_kernel_spmd

    in_maps = make_in_maps(inputs)
    nc = _get_nc()
    res = run_bass_kernel_spmd(nc, in_maps, list(range(NCORES)))
    out = np.concatenate([res.results[c]["out"] for c in range(NCORES)], axis=0)
    applied = np.concatenate(
        [np.ascontiguousarray(res.results[c]["appliedT"].T)
         for c in range(NCORES)], axis=0)
    return out.astype(np.float32), applied.astype(np.float32)


# revision 23
# speedup vs baseline: 1.1090x; 1.1090x over previous
"""Trainium2 Bass kernel for nn_AttentionModule (Bahdanau-style attention).

Reference computation (S=512, B=64, H=1024, F=2H):
    cat    = concat([hidden bcast to (S,B,H), encoder_states], -1)      [S,B,2H]
    scores = tanh(cat @ W_attn.T + b_attn) @ W_attn2.T + b_attn2        [S,B,1]
    attn   = softmax(scores[..., 0].T, axis=-1)                         [B,S]
    applied= einsum("bs,sbh->bh", attn, encoder_states)                 [B,H]
    out    = tanh(concat([decoder_out, applied], -1) @ W_comb.T + b_comb)

Sharding: data-parallel over B across 8 cores (8 batch rows per core).

v4 structure:
  - Main matmul T^T[f,s] = sum_h W2T[h,f]*encT[h,s] in fp8e4m3 DoubleRow
    (256 contraction rows per instruction).  W2 host-scaled by 256; the
    tanh undoes it via its fused scale=1/256.  W2T is laid out per-f-tile
    so the first DR group only waits on a 128KB DMA.
  - The first four DR groups of batch row 0 are emitted BEFORE the
    preamble so the PE has work while W1 streams in.
  - hid@W1.T preamble: W1 resident [128, KH*F], 16x256KB DMAs issued on
    the (idle) scalar queue in consumption order; 32 bf16 matmuls + PE
    transposes.  Scores bias hidb folded into tanh as per-partition bias.
  - Scores matmul (attn2) in bf16 with W_attn2 replicated to 128 columns:
    the psum scores tile [128, S] carries the row broadcast for free
    (fp8 tanh outputs measurably hurt the attention weights, so attn2
    stays bf16).
  - Softmax skips max-subtraction (scores are provably tiny); Exp fuses
    the sum via accum_out.
  - applied^T: bf16 mult+reduce per h-chunk on VectorE over a bf16
    encoder copy (gpsimd is too slow per-op to help).
  - enc fp8/bf16 copies streamed per batch row (prefetch one ahead).
  - Final combine bf16; decoder half emitted before the last batch row
    so the PE tail only waits on the last row's applied.
"""

import numpy as np

S, B, H = 512, 64, 1024
F = 2 * H
NCORES = 8
BL = B // NCORES          # 8 batch rows per core
KH = H // 128             # 8 contraction chunks over H
KF = F // 128             # 16 feature tiles
W2SCALE = 256.0           # host pre-scale on W2 for fp8 range

_CACHE = {}


def _build(num_devices=NCORES):
    from contextlib import ExitStack

    import concourse.tile as tile
    from concourse import bacc, mybir
    from concourse.masks import make_identity

    f32 = mybir.dt.float32
    bf16 = mybir.dt.bfloat16
    f8 = mybir.dt.float8e4
    AF = mybir.ActivationFunctionType
    ALU = mybir.AluOpType
    AX = mybir.AxisListType
    DR = mybir.MatmulPerfMode.DoubleRow

    nc = bacc.Bacc("TRN2", target_bir_lowering=False, debug=False,
                   num_devices=num_devices)

    # enc free layout per partition: [b, kc, s]; h = kc*128 + p
    enc8_d = nc.dram_tensor("enc8", [128, BL * KH * S], f8,
                            kind="ExternalInput").ap()
    encb_d = nc.dram_tensor("encb", [128, BL * KH * S], bf16,
                            kind="ExternalInput").ap()
    # w2t8 free layout: [ft, kc, f]; wat1 free layout: [kc, f]
    w2t8_d = nc.dram_tensor("w2t8", [128, KF * KH * 128], f8,
                            kind="ExternalInput").ap()
    wat1_d = nc.dram_tensor("wat1", [128, KH * F], bf16,
                            kind="ExternalInput").ap()
    wct = nc.dram_tensor("wct", [F, H], bf16, kind="ExternalInput").ap()
    hidT = nc.dram_tensor("hidT", [H, BL], f32, kind="ExternalInput").ap()
    decT = nc.dram_tensor("decT", [H, BL], f32, kind="ExternalInput").ap()
    w2rep = nc.dram_tensor("w2rep", [F, 128], f32, kind="ExternalInput").ap()
    b_attn_d = nc.dram_tensor("b_attn", [1, F], f32, kind="ExternalInput").ap()
    b_comb_d = nc.dram_tensor("b_comb", [1, H], f32, kind="ExternalInput").ap()
    out_d = nc.dram_tensor("out", [BL, H], f32, kind="ExternalOutput").ap()
    appT_d = nc.dram_tensor("appliedT", [H, BL], f32,
                            kind="ExternalOutput").ap()

    with tile.TileContext(nc) as tc:
        with ExitStack() as ctx:
            consts = ctx.enter_context(tc.tile_pool(name="consts", bufs=1))
            enc8_pool = ctx.enter_context(tc.tile_pool(name="enc8p", bufs=2))
            encb_pool = ctx.enter_context(tc.tile_pool(name="encbp", bufs=2))
            tanh_pool = ctx.enter_context(tc.tile_pool(name="tanh", bufs=18))
            attn_pool = ctx.enter_context(tc.tile_pool(name="attn", bufs=2))
            scr_pool = ctx.enter_context(tc.tile_pool(name="scr", bufs=2))
            small_pool = ctx.enter_context(tc.tile_pool(name="small", bufs=4))
            wct_pool = ctx.enter_context(tc.tile_pool(name="wct", bufs=4))
            psT_pool = ctx.enter_context(
                tc.tile_pool(name="psT", bufs=4, space="PSUM"))
            psSc_pool = ctx.enter_context(
                tc.tile_pool(name="psSc", bufs=2, space="PSUM"))
            psPre_pool = ctx.enter_context(
                tc.tile_pool(name="psPre", bufs=2, space="PSUM"))

            # ---- W1 on the scalar queue (idle early), consumption order ----
            w1_sb = consts.tile([128, KH * F], bf16)
            for j in range(2):
                for kc in range(KH):
                    nc.scalar.dma_start(
                        w1_sb[:, kc * F + j * 1024: kc * F + (j + 1) * 1024],
                        wat1_d[:, kc * F + j * 1024: kc * F + (j + 1) * 1024])

            # ---- sync queue: W2 head chunk, enc b0, then the rest ----
            w2t8_sb = consts.tile([128, KF * KH * 128], f8)
            CW = KH * 128

            def w2_load(ft):
                nc.sync.dma_start(w2t8_sb[:, ft * CW:(ft + 1) * CW],
                                  w2t8_d[:, ft * CW:(ft + 1) * CW])

            def enc8_load(b):
                t = enc8_pool.tile([128, KH * S], f8, tag="enc8", name="enc8")
                half = KH * S // 2
                for i in range(2):
                    nc.sync.dma_start(
                        t[:, i * half:(i + 1) * half],
                        enc8_d[:, b * KH * S + i * half:
                               b * KH * S + (i + 1) * half])
                return t

            def encb_load(b):
                t = encb_pool.tile([128, KH * S], bf16, tag="encb",
                                   name="encb")
                q = KH * S // 4
                for i in range(4):
                    nc.sync.dma_start(
                        t[:, i * q:(i + 1) * q],
                        encb_d[:, b * KH * S + i * q:
                               b * KH * S + (i + 1) * q])
                return t

            w2_load(0)
            enc8_tiles = {0: enc8_load(0)}
            for ft in range(1, 4):
                w2_load(ft)

            # ---- small constants (shipped fp32, cast on device) ----
            identity = consts.tile([128, 128], f32)
            make_identity(nc, identity[:])
            ones_bf = consts.tile([1, BL], bf16)
            nc.vector.memset(ones_bf[:], 1.0)
            hidT_32 = consts.tile([128, KH * BL], f32)
            for kc in range(KH):
                nc.sync.dma_start(hidT_32[:, kc * BL:(kc + 1) * BL],
                                  hidT[kc * 128:(kc + 1) * 128, :])
            hidT_sb = consts.tile([128, KH * BL], bf16)
            nc.vector.tensor_copy(hidT_sb[:], hidT_32[:])
            b_attn_32 = consts.tile([1, F], f32)
            nc.sync.dma_start(b_attn_32[:], b_attn_d[:])
            b_attn_sb = consts.tile([1, F], bf16)
            nc.vector.tensor_copy(b_attn_sb[:], b_attn_32[:])
            w2rep_32 = consts.tile([128, KF * 128], f32)
            for i in range(4):
                nc.sync.dma_start(
                    w2rep_32[:, i * 512:(i + 1) * 512].rearrange(
                        "p (a c) -> p a c", a=4),
                    w2rep[i * 512:(i + 1) * 512, :].rearrange(
                        "(a p) c -> p a c", p=128))
            w2rep_sb = consts.tile([128, KF * 128], bf16)
            nc.vector.tensor_copy(w2rep_sb[:], w2rep_32[:])

            for ft in range(4, KF):
                w2_load(ft)

            decT_32 = consts.tile([128, KH * BL], f32)
            for kc in range(KH):
                nc.sync.dma_start(decT_32[:, kc * BL:(kc + 1) * BL],
                                  decT[kc * 128:(kc + 1) * 128, :])
            decT_sb = consts.tile([128, KH * BL], bf16)
            nc.vector.tensor_copy(decT_sb[:], decT_32[:])
            b_comb_32 = consts.tile([1, H], f32)
            nc.sync.dma_start(b_comb_32[:], b_comb_d[:])
            b_comb_sb = consts.tile([1, H], bf16)
            nc.vector.tensor_copy(b_comb_sb[:], b_comb_32[:])
            encb_tiles = {0: encb_load(0)}

            hidbT_sb = consts.tile([128, KF * BL], f32)
            appT_sb = consts.tile([128, KH * BL], f32)
            appT_bf = consts.tile([128, KH * BL], bf16)
            w2t8_v = w2t8_sb.rearrange("p (t k f) -> p t k f", t=KF, k=KH)

            # ---- prologue: first 4 DR groups of b0 keep the PE busy while
            # W1 streams in for the preamble ----
            def dr_group(et8_v, ft):
                pT = psT_pool.tile([128, S], f32, tag="pT", name="pT")
                for kc2 in range(KH // 2):
                    nc.tensor.matmul(
                        pT[:],
                        w2t8_v[:, ft, 2 * kc2:2 * kc2 + 2, :],
                        et8_v[:, 2 * kc2:2 * kc2 + 2, :],
                        start=(kc2 == 0), stop=(kc2 == KH // 2 - 1),
                        perf_mode=DR)
                return pT

            et8_b0 = enc8_tiles.pop(0)
            et8_b0_v = et8_b0.rearrange("p (k s) -> p k s", k=KH)
            pT_pending = [dr_group(et8_b0_v, ft) for ft in range(4)]

            # ---- hid_part preamble: hidb[b, f] = hidden @ W1.T + b_attn ----
            hidb_row = consts.tile([BL, F], f32)
            for fc in range(F // 512):
                ph = psPre_pool.tile([BL, 512], f32, tag="pre", name=f"ph{fc}")
                for kc in range(KH):
                    nc.tensor.matmul(
                        ph[:], hidT_sb[:, kc * BL:(kc + 1) * BL],
                        w1_sb[:, kc * F + fc * 512: kc * F + (fc + 1) * 512],
                        start=(kc == 0), stop=False)
                nc.tensor.matmul(
                    ph[:], ones_bf[:], b_attn_sb[:, fc * 512:(fc + 1) * 512],
                    start=False, stop=True)
                nc.vector.tensor_copy(hidb_row[:, fc * 512:(fc + 1) * 512],
                                      ph[:])
            for ft in range(KF):
                ptp = psPre_pool.tile([128, BL], f32, tag="pre", name="ptp")
                nc.tensor.transpose(ptp[:],
                                    hidb_row[:, ft * 128:(ft + 1) * 128],
                                    identity[:BL, :BL])
                nc.vector.tensor_copy(hidbT_sb[:, ft * BL:(ft + 1) * BL],
                                      ptp[:])

            # ---- main loop over local batch rows ----
            inv = 1.0 / W2SCALE
            pouts = [None, None]
            for b in range(BL):
                if b + 1 < BL:
                    enc8_tiles[b + 1] = enc8_load(b + 1)
                    encb_tiles[b + 1] = encb_load(b + 1)
                if b > 0:
                    et8 = enc8_tiles.pop(b)
                    et8_v = et8.rearrange("p (k s) -> p k s", k=KH)
                else:
                    et8_v = et8_b0_v

                psc = psSc_pool.tile([128, S], f32, tag="psc", name="psc")
                th = [None] * KF

                def attn2(ft):
                    nc.tensor.matmul(
                        psc[:],
                        w2rep_sb[:, ft * 128:(ft + 1) * 128],
                        th[ft][:],
                        start=(ft == 0), stop=(ft == KF - 1))

                for ft in range(KF):
                    if b == 0 and ft < 4:
                        pT = pT_pending[ft]
                    else:
                        pT = dr_group(et8_v, ft)
                    t = tanh_pool.tile([128, S], bf16, tag="tanh", name="tanh")
                    nc.scalar.activation(
                        t[:], pT[:], AF.Tanh,
                        bias=hidbT_sb[:, ft * BL + b: ft * BL + b + 1],
                        scale=inv)
                    th[ft] = t
                    if ft >= 2:
                        attn2(ft - 2)
                attn2(KF - 2)
                attn2(KF - 1)

                # softmax over s (no max subtraction: |scores| <~ 2)
                attn = attn_pool.tile([128, S], bf16, tag="attn", name="attn")
                sumexp = small_pool.tile([128, 1], f32, tag="sumexp",
                                         name="sumexp")
                nc.scalar.activation(attn[:], psc[:], AF.Exp,
                                     bias=0.0, scale=1.0,
                                     accum_out=sumexp[:])
                recip = small_pool.tile([128, 1], f32, tag="recip",
                                        name="recip")
                nc.vector.reciprocal(recip[:], sumexp[:])
                nc.vector.tensor_scalar_mul(attn[:], attn[:], recip[:])

                # applied^T[h, b]: bf16 mult+reduce on VectorE
                etb = encb_tiles.pop(b)
                etb_v = etb.rearrange("p (k s) -> p k s", k=KH)
                for kc in range(KH):
                    scr = scr_pool.tile([128, S], bf16, tag="scr", name="scr")
                    nc.vector.tensor_tensor(out=scr[:], in0=etb_v[:, kc, :],
                                            in1=attn[:], op=ALU.mult)
                    nc.vector.reduce_sum(
                        appT_sb[:, kc * BL + b: kc * BL + b + 1],
                        scr[:], axis=AX.X)

                # decoder half of the final combine before the last row
                if b == BL - 2:
                    for i in range(2):
                        pouts[i] = psPre_pool.tile([BL, 512], f32, tag="pre",
                                                   name=f"po{i}")
                    for kc in range(KH):
                        w = wct_pool.tile([128, H], bf16, tag="wct",
                                          name="wctt")
                        nc.sync.dma_start(w[:], wct[kc * 128:(kc + 1) * 128, :])
                        for fc in range(2):
                            nc.tensor.matmul(
                                pouts[fc][:],
                                decT_sb[:, kc * BL:(kc + 1) * BL],
                                w[:, fc * 512:(fc + 1) * 512],
                                start=(kc == 0), stop=False)

            # ---- final combine: += applied @ Wc_applied.T, bias, tanh ----
            nc.vector.tensor_copy(appT_bf[:], appT_sb[:])
            for kc in range(KH):
                w = wct_pool.tile([128, H], bf16, tag="wct", name="wctt")
                nc.sync.dma_start(w[:], wct[(KH + kc) * 128:
                                            (KH + kc + 1) * 128, :])
                for fc in range(2):
                    nc.tensor.matmul(
                        pouts[fc][:], appT_bf[:, kc * BL:(kc + 1) * BL],
                        w[:, fc * 512:(fc + 1) * 512],
                        start=False, stop=False)
            for fc in range(2):
                nc.tensor.matmul(
                    pouts[fc][:], ones_bf[:],
                    b_comb_sb[:, fc * 512:(fc + 1) * 512],
                    start=False, stop=True)

            out_sb = consts.tile([BL, H], f32)
            for fc in range(2):
                nc.scalar.activation(out_sb[:, fc * 512:(fc + 1) * 512],
                                     pouts[fc][:], AF.Tanh)
            nc.sync.dma_start(out_d[:], out_sb[:])
            for kc in range(KH):
                nc.sync.dma_start(appT_d[kc * 128:(kc + 1) * 128, :],
                                  appT_sb[:, kc * BL:(kc + 1) * BL])

    nc.compile()
    return nc


def _get_nc():
    if "nc" not in _CACHE:
        _CACHE["nc"] = _build()
    return _CACHE["nc"]


def make_in_maps(inputs):
    import ml_dtypes
    bf = ml_dtypes.bfloat16
    f8 = ml_dtypes.float8_e4m3

    inp = {k: np.asarray(v, dtype=np.float32) for k, v in inputs.items()}
    hidden = inp["hidden"]
    decoder_out = inp["decoder_out"]
    encoder_states = inp["encoder_states"]
    W_attn = inp["W_attn"]
    b_attn = inp["b_attn"]
    W_attn2 = inp["W_attn2"]
    W_comb = inp["W_comb"]
    b_comb = inp["b_comb"]
    # b_attn2 shifts every score equally -> softmax-invariant, unused.

    wat = np.ascontiguousarray(W_attn.T)                    # [F, F] fp32
    wat1 = np.ascontiguousarray(
        wat[:H].reshape(KH, 128, F).transpose(1, 0, 2)
        .reshape(128, KH * F)).astype(bf)
    w2t8 = np.ascontiguousarray(
        (wat[H:] * W2SCALE).reshape(KH, 128, KF, 128)
        .transpose(1, 2, 0, 3).reshape(128, KF * KH * 128)).astype(f8)
    wct = np.ascontiguousarray(W_comb.T).astype(bf)
    w2rep = np.ascontiguousarray(np.repeat(W_attn2.reshape(F, 1), 128, axis=1))
    b_attn_2d = np.ascontiguousarray(b_attn.reshape(1, F))
    b_comb_2d = np.ascontiguousarray(b_comb.reshape(1, H))

    in_maps = []
    for c in range(NCORES):
        sl = slice(c * BL, (c + 1) * BL)
        # [S, BL, H] -> [BL, H, S] -> [BL, KH, 128, S] -> [128, BL, KH, S]
        enc = np.ascontiguousarray(
            encoder_states[:, sl, :].transpose(1, 2, 0)
            .reshape(BL, KH, 128, S).transpose(2, 0, 1, 3)
            .reshape(128, BL * KH * S))
        in_maps.append({
            "enc8": enc.astype(f8),
            "encb": enc.astype(bf),
            "w2t8": w2t8,
            "wat1": wat1,
            "wct": wct,
            "hidT": np.ascontiguousarray(hidden[sl].T),
            "decT": np.ascontiguousarray(decoder_out[sl].T),
            "w2rep": w2rep,
            "b_attn": b_attn_2d,
            "b_comb": b_comb_2d,
        })
    return in_maps


def kernel(**inputs):
    from concourse.bass_utils import run_bass_kernel_spmd

    in_maps = make_in_maps(inputs)
    nc = _get_nc()
    res = run_bass_kernel_spmd(nc, in_maps, list(range(NCORES)))
    out = np.concatenate([res.results[c]["out"] for c in range(NCORES)], axis=0)
    applied = np.concatenate(
        [np.ascontiguousarray(res.results[c]["appliedT"].T)
         for c in range(NCORES)], axis=0)
    return out.astype(np.float32), applied.astype(np.float32)


# revision 28
# speedup vs baseline: 1.1846x; 1.0681x over previous
"""Trainium2 Bass kernel for nn_AttentionModule (Bahdanau-style attention).

Reference computation (S=512, B=64, H=1024, F=2H):
    cat    = concat([hidden bcast to (S,B,H), encoder_states], -1)      [S,B,2H]
    scores = tanh(cat @ W_attn.T + b_attn) @ W_attn2.T + b_attn2        [S,B,1]
    attn   = softmax(scores[..., 0].T, axis=-1)                         [B,S]
    applied= einsum("bs,sbh->bh", attn, encoder_states)                 [B,H]
    out    = tanh(concat([decoder_out, applied], -1) @ W_comb.T + b_comb)

Sharding: data-parallel over B across 8 cores (8 batch rows per core).

v4 structure:
  - Main matmul T^T[f,s] = sum_h W2T[h,f]*encT[h,s] in fp8e4m3 DoubleRow
    (256 contraction rows per instruction).  W2 host-scaled by 256; the
    tanh undoes it via its fused scale=1/256.  W2T is laid out per-f-tile
    so the first DR group only waits on a 128KB DMA.
  - The first four DR groups of batch row 0 are emitted BEFORE the
    preamble so the PE has work while W1 streams in.
  - hid@W1.T preamble: W1 resident [128, KH*F], 16x256KB DMAs issued on
    the (idle) scalar queue in consumption order; 32 bf16 matmuls + PE
    transposes.  Scores bias hidb folded into tanh as per-partition bias.
  - Scores matmul (attn2) in bf16 with W_attn2 replicated to 128 columns:
    the psum scores tile [128, S] carries the row broadcast for free
    (fp8 tanh outputs measurably hurt the attention weights, so attn2
    stays bf16).
  - Softmax skips max-subtraction (scores are provably tiny); Exp fuses
    the sum via accum_out.
  - applied^T: bf16 mult+reduce per h-chunk on VectorE over a bf16
    encoder copy (gpsimd is too slow per-op to help).
  - enc fp8/bf16 copies streamed per batch row (prefetch one ahead).
  - Final combine bf16; decoder half emitted before the last batch row
    so the PE tail only waits on the last row's applied.
"""

import numpy as np

S, B, H = 512, 64, 1024
F = 2 * H
NCORES = 8
BL = B // NCORES          # 8 batch rows per core
KH = H // 128             # 8 contraction chunks over H
KF = F // 128             # 16 feature tiles
W2SCALE = 256.0           # host pre-scale on W2 for fp8 range

_CACHE = {}


def _build(num_devices=NCORES):
    from contextlib import ExitStack

    import concourse.tile as tile
    from concourse import bacc, mybir
    from concourse.masks import make_identity

    f32 = mybir.dt.float32
    bf16 = mybir.dt.bfloat16
    f8 = mybir.dt.float8e4
    AF = mybir.ActivationFunctionType
    ALU = mybir.AluOpType
    AX = mybir.AxisListType
    DR = mybir.MatmulPerfMode.DoubleRow

    nc = bacc.Bacc("TRN2", target_bir_lowering=False, debug=False,
                   num_devices=num_devices)

    # enc free layout per partition: [b, kc, s]; h = kc*128 + p
    enc8_d = nc.dram_tensor("enc8", [128, BL * KH * S], f8,
                            kind="ExternalInput").ap()
    encb_d = nc.dram_tensor("encb", [128, BL * KH * S], bf16,
                            kind="ExternalInput").ap()
    # w2t8 free layout: [ft, kc, f]; wat1 free layout: [kc, f]
    w2t8_d = nc.dram_tensor("w2t8", [128, KF * KH * 128], f8,
                            kind="ExternalInput").ap()
    wat1_d = nc.dram_tensor("wat1", [128, KH * F], bf16,
                            kind="ExternalInput").ap()
    wct = nc.dram_tensor("wct", [F, H], bf16, kind="ExternalInput").ap()
    hidT = nc.dram_tensor("hidT", [H, BL], f32, kind="ExternalInput").ap()
    decT = nc.dram_tensor("decT", [H, BL], f32, kind="ExternalInput").ap()
    # [128, (ft c)] fp8, value 256*W_attn2[ft*128+p] replicated over c
    w2rep = nc.dram_tensor("w2rep", [128, KF * 128], f8,
                           kind="ExternalInput").ap()
    b_attn_d = nc.dram_tensor("b_attn", [1, F], f32, kind="ExternalInput").ap()
    b_comb_d = nc.dram_tensor("b_comb", [1, H], f32, kind="ExternalInput").ap()
    out_d = nc.dram_tensor("out", [BL, H], f32, kind="ExternalOutput").ap()
    appT_d = nc.dram_tensor("appliedT", [H, BL], f32,
                            kind="ExternalOutput").ap()

    with tile.TileContext(nc) as tc:
        with ExitStack() as ctx:
            consts = ctx.enter_context(tc.tile_pool(name="consts", bufs=1))
            enc8_pool = ctx.enter_context(tc.tile_pool(name="enc8p", bufs=2))
            encb_pool = ctx.enter_context(tc.tile_pool(name="encbp", bufs=2))
            tanh_pool = ctx.enter_context(tc.tile_pool(name="tanh", bufs=18))
            attn_pool = ctx.enter_context(tc.tile_pool(name="attn", bufs=2))
            scr_pool = ctx.enter_context(tc.tile_pool(name="scr", bufs=2))
            small_pool = ctx.enter_context(tc.tile_pool(name="small", bufs=4))
            wct_pool = ctx.enter_context(tc.tile_pool(name="wct", bufs=4))
            psT_pool = ctx.enter_context(
                tc.tile_pool(name="psT", bufs=4, space="PSUM"))
            psSc_pool = ctx.enter_context(
                tc.tile_pool(name="psSc", bufs=2, space="PSUM"))
            psPre_pool = ctx.enter_context(
                tc.tile_pool(name="psPre", bufs=2, space="PSUM"))

            # ---- W1 on the scalar queue (idle early), consumption order ----
            w1_sb = consts.tile([128, KH * F], bf16)
            for j in range(2):
                for kc in range(KH):
                    nc.scalar.dma_start(
                        w1_sb[:, kc * F + j * 1024: kc * F + (j + 1) * 1024],
                        wat1_d[:, kc * F + j * 1024: kc * F + (j + 1) * 1024])

            # ---- sync queue: W2 head chunk, enc b0, then the rest ----
            w2t8_sb = consts.tile([128, KF * KH * 128], f8)
            CW = KH * 128

            def w2_load(ft):
                nc.sync.dma_start(w2t8_sb[:, ft * CW:(ft + 1) * CW],
                                  w2t8_d[:, ft * CW:(ft + 1) * CW])

            def enc8_load(b):
                t = enc8_pool.tile([128, KH * S], f8, tag="enc8", name="enc8")
                half = KH * S // 2
                for i in range(2):
                    nc.sync.dma_start(
                        t[:, i * half:(i + 1) * half],
                        enc8_d[:, b * KH * S + i * half:
                               b * KH * S + (i + 1) * half])
                return t

            def encb_load(b):
                t = encb_pool.tile([128, KH * S], bf16, tag="encb",
                                   name="encb")
                q = KH * S // 4
                for i in range(4):
                    nc.sync.dma_start(
                        t[:, i * q:(i + 1) * q],
                        encb_d[:, b * KH * S + i * q:
                               b * KH * S + (i + 1) * q])
                return t

            w2_load(0)
            enc8_tiles = {0: enc8_load(0)}
            for ft in range(1, 4):
                w2_load(ft)

            # ---- small constants (shipped fp32, cast on device) ----
            identity = consts.tile([128, 128], f32)
            make_identity(nc, identity[:])
            ones_bf = consts.tile([1, BL], bf16)
            nc.vector.memset(ones_bf[:], 1.0)
            hidT_32 = consts.tile([128, KH * BL], f32)
            for kc in range(KH):
                nc.sync.dma_start(hidT_32[:, kc * BL:(kc + 1) * BL],
                                  hidT[kc * 128:(kc + 1) * 128, :])
            hidT_sb = consts.tile([128, KH * BL], bf16)
            nc.vector.tensor_copy(hidT_sb[:], hidT_32[:])
            b_attn_32 = consts.tile([1, F], f32)
            nc.sync.dma_start(b_attn_32[:], b_attn_d[:])
            b_attn_sb = consts.tile([1, F], bf16)
            nc.vector.tensor_copy(b_attn_sb[:], b_attn_32[:])
            w2rep8_sb = consts.tile([128, KF * 128], f8)
            nc.sync.dma_start(w2rep8_sb[:], w2rep[:])
            w2rep8_v = w2rep8_sb.rearrange("p (t c) -> p t c", t=KF)

            for ft in range(4, KF):
                w2_load(ft)

            decT_32 = consts.tile([128, KH * BL], f32)
            for kc in range(KH):
                nc.sync.dma_start(decT_32[:, kc * BL:(kc + 1) * BL],
                                  decT[kc * 128:(kc + 1) * 128, :])
            decT_sb = consts.tile([128, KH * BL], bf16)
            nc.vector.tensor_copy(decT_sb[:], decT_32[:])
            b_comb_32 = consts.tile([1, H], f32)
            nc.sync.dma_start(b_comb_32[:], b_comb_d[:])
            b_comb_sb = consts.tile([1, H], bf16)
            nc.vector.tensor_copy(b_comb_sb[:], b_comb_32[:])
            encb_tiles = {0: encb_load(0)}

            hidbT_sb = consts.tile([128, KF * BL], f32)
            appT_sb = consts.tile([128, KH * BL], f32)
            appT_bf = consts.tile([128, KH * BL], bf16)
            w2t8_v = w2t8_sb.rearrange("p (t k f) -> p t k f", t=KF, k=KH)

            # ---- prologue: first 4 DR groups of b0 keep the PE busy while
            # W1 streams in for the preamble ----
            def dr_group(et8_v, ft):
                pT = psT_pool.tile([128, S], f32, tag="pT", name="pT")
                for kc2 in range(KH // 2):
                    nc.tensor.matmul(
                        pT[:],
                        w2t8_v[:, ft, 2 * kc2:2 * kc2 + 2, :],
                        et8_v[:, 2 * kc2:2 * kc2 + 2, :],
                        start=(kc2 == 0), stop=(kc2 == KH // 2 - 1),
                        perf_mode=DR)
                return pT

            et8_b0 = enc8_tiles.pop(0)
            et8_b0_v = et8_b0.rearrange("p (k s) -> p k s", k=KH)
            pT_pending = [dr_group(et8_b0_v, ft) for ft in range(4)]

            # ---- hid_part preamble: hidb[b, f] = hidden @ W1.T + b_attn ----
            hidb_row = consts.tile([BL, F], f32)
            for fc in range(F // 512):
                ph = psPre_pool.tile([BL, 512], f32, tag="pre", name=f"ph{fc}")
                for kc in range(KH):
                    nc.tensor.matmul(
                        ph[:], hidT_sb[:, kc * BL:(kc + 1) * BL],
                        w1_sb[:, kc * F + fc * 512: kc * F + (fc + 1) * 512],
                        start=(kc == 0), stop=False)
                nc.tensor.matmul(
                    ph[:], ones_bf[:], b_attn_sb[:, fc * 512:(fc + 1) * 512],
                    start=False, stop=True)
                nc.vector.tensor_copy(hidb_row[:, fc * 512:(fc + 1) * 512],
                                      ph[:])
            for ft in range(KF):
                ptp = psPre_pool.tile([128, BL], f32, tag="pre", name="ptp")
                nc.tensor.transpose(ptp[:],
                                    hidb_row[:, ft * 128:(ft + 1) * 128],
                                    identity[:BL, :BL])
                nc.vector.tensor_copy(hidbT_sb[:, ft * BL:(ft + 1) * BL],
                                      ptp[:])

            # ---- main loop over local batch rows ----
            inv = 1.0 / W2SCALE
            pouts = [None, None]
            for b in range(BL):
                if b + 1 < BL:
                    enc8_tiles[b + 1] = enc8_load(b + 1)
                    encb_tiles[b + 1] = encb_load(b + 1)
                if b > 0:
                    et8 = enc8_tiles.pop(b)
                    et8_v = et8.rearrange("p (k s) -> p k s", k=KH)
                else:
                    et8_v = et8_b0_v

                psc = psSc_pool.tile([128, S], f32, tag="psc", name="psc")
                thp = [None] * (KF // 2)

                for ft in range(KF):
                    if b == 0 and ft < 4:
                        pT = pT_pending[ft]
                    else:
                        pT = dr_group(et8_v, ft)
                    if ft % 2 == 0:
                        thp[ft // 2] = tanh_pool.tile([128, 2 * S], f8,
                                                      tag="tanh", name="tanh")
                    nc.scalar.activation(
                        thp[ft // 2][:, (ft % 2) * S:(ft % 2 + 1) * S],
                        pT[:], AF.Tanh,
                        bias=hidbT_sb[:, ft * BL + b: ft * BL + b + 1],
                        scale=inv)
                # scores matmul as ONE consecutive fp8 DoubleRow group (the
                # lag-interleaved emission of this group miscomputes on HW)
                for fp in range(KF // 2):
                    nc.tensor.matmul(
                        psc[:], w2rep8_v[:, 2 * fp:2 * fp + 2, :],
                        thp[fp].rearrange("p (t s) -> p t s", t=2),
                        start=(fp == 0), stop=(fp == KF // 2 - 1),
                        perf_mode=DR)

                # softmax over s (no max subtraction: |scores| <~ 2)
                attn = attn_pool.tile([128, S], bf16, tag="attn", name="attn")
                sumexp = small_pool.tile([128, 1], f32, tag="sumexp",
                                         name="sumexp")
                nc.scalar.activation(attn[:], psc[:], AF.Exp,
                                     bias=0.0, scale=inv,
                                     accum_out=sumexp[:])
                recip = small_pool.tile([128, 1], f32, tag="recip",
                                        name="recip")
                nc.vector.reciprocal(recip[:], sumexp[:])
                nc.vector.tensor_scalar_mul(attn[:], attn[:], recip[:])

                # applied^T[h, b]: bf16 mult+reduce on VectorE
                etb = encb_tiles.pop(b)
                etb_v = etb.rearrange("p (k s) -> p k s", k=KH)
                for kc in range(KH):
                    scr = scr_pool.tile([128, S], bf16, tag="scr", name="scr")
                    nc.vector.tensor_tensor(out=scr[:], in0=etb_v[:, kc, :],
                                            in1=attn[:], op=ALU.mult)
                    nc.vector.reduce_sum(
                        appT_sb[:, kc * BL + b: kc * BL + b + 1],
                        scr[:], axis=AX.X)

                # decoder half of the final combine before the last row
                if b == BL - 2:
                    for i in range(2):
                        pouts[i] = psPre_pool.tile([BL, 512], f32, tag="pre",
                                                   name=f"po{i}")
                    for kc in range(KH):
                        w = wct_pool.tile([128, H], bf16, tag="wct",
                                          name="wctt")
                        nc.sync.dma_start(w[:], wct[kc * 128:(kc + 1) * 128, :])
                        for fc in range(2):
                            nc.tensor.matmul(
                                pouts[fc][:],
                                decT_sb[:, kc * BL:(kc + 1) * BL],
                                w[:, fc * 512:(fc + 1) * 512],
                                start=(kc == 0), stop=False)

            # ---- final combine: += applied @ Wc_applied.T, bias, tanh ----
            nc.vector.tensor_copy(appT_bf[:], appT_sb[:])
            for kc in range(KH):
                w = wct_pool.tile([128, H], bf16, tag="wct", name="wctt")
                nc.sync.dma_start(w[:], wct[(KH + kc) * 128:
                                            (KH + kc + 1) * 128, :])
                for fc in range(2):
                    nc.tensor.matmul(
                        pouts[fc][:], appT_bf[:, kc * BL:(kc + 1) * BL],
                        w[:, fc * 512:(fc + 1) * 512],
                        start=False, stop=False)
            for fc in range(2):
                nc.tensor.matmul(
                    pouts[fc][:], ones_bf[:],
                    b_comb_sb[:, fc * 512:(fc + 1) * 512],
                    start=False, stop=True)

            out_sb = consts.tile([BL, H], f32)
            for fc in range(2):
                nc.scalar.activation(out_sb[:, fc * 512:(fc + 1) * 512],
                                     pouts[fc][:], AF.Tanh)
            nc.sync.dma_start(out_d[:], out_sb[:])
            for kc in range(KH):
                nc.sync.dma_start(appT_d[kc * 128:(kc + 1) * 128, :],
                                  appT_sb[:, kc * BL:(kc + 1) * BL])

    nc.compile()
    return nc


def _get_nc():
    if "nc" not in _CACHE:
        _CACHE["nc"] = _build()
    return _CACHE["nc"]


def make_in_maps(inputs):
    import ml_dtypes
    bf = ml_dtypes.bfloat16
    f8 = ml_dtypes.float8_e4m3

    inp = {k: np.asarray(v, dtype=np.float32) for k, v in inputs.items()}
    hidden = inp["hidden"]
    decoder_out = inp["decoder_out"]
    encoder_states = inp["encoder_states"]
    W_attn = inp["W_attn"]
    b_attn = inp["b_attn"]
    W_attn2 = inp["W_attn2"]
    W_comb = inp["W_comb"]
    b_comb = inp["b_comb"]
    # b_attn2 shifts every score equally -> softmax-invariant, unused.

    wat = np.ascontiguousarray(W_attn.T)                    # [F, F] fp32
    wat1 = np.ascontiguousarray(
        wat[:H].reshape(KH, 128, F).transpose(1, 0, 2)
        .reshape(128, KH * F)).astype(bf)
    w2t8 = np.ascontiguousarray(
        (wat[H:] * W2SCALE).reshape(KH, 128, KF, 128)
        .transpose(1, 2, 0, 3).reshape(128, KF * KH * 128)).astype(f8)
    wct = np.ascontiguousarray(W_comb.T).astype(bf)
    w2r = (W2SCALE * W_attn2.reshape(KF, 128).T).astype(np.float32)
    w2rep = np.ascontiguousarray(
        np.broadcast_to(w2r[:, :, None], (128, KF, 128))
        .reshape(128, KF * 128)).astype(f8)
    b_attn_2d = np.ascontiguousarray(b_attn.reshape(1, F))
    b_comb_2d = np.ascontiguousarray(b_comb.reshape(1, H))

    in_maps = []
    for c in range(NCORES):
        sl = slice(c * BL, (c + 1) * BL)
        # [S, BL, H] -> [BL, H, S] -> [BL, KH, 128, S] -> [128, BL, KH, S]
        enc = np.ascontiguousarray(
            encoder_states[:, sl, :].transpose(1, 2, 0)
            .reshape(BL, KH, 128, S).transpose(2, 0, 1, 3)
            .reshape(128, BL * KH * S))
        in_maps.append({
            "enc8": enc.astype(f8),
            "encb": enc.astype(bf),
            "w2t8": w2t8,
            "wat1": wat1,
            "wct": wct,
            "hidT": np.ascontiguousarray(hidden[sl].T),
            "decT": np.ascontiguousarray(decoder_out[sl].T),
            "w2rep": w2rep,
            "b_attn": b_attn_2d,
            "b_comb": b_comb_2d,
        })
    return in_maps


def kernel(**inputs):
    from concourse.bass_utils import run_bass_kernel_spmd

    in_maps = make_in_maps(inputs)
    nc = _get_nc()
    res = run_bass_kernel_spmd(nc, in_maps, list(range(NCORES)))
    out = np.concatenate([res.results[c]["out"] for c in range(NCORES)], axis=0)
    applied = np.concatenate(
        [np.ascontiguousarray(res.results[c]["appliedT"].T)
         for c in range(NCORES)], axis=0)
    return out.astype(np.float32), applied.astype(np.float32)


# revision 34
# speedup vs baseline: 1.2065x; 1.0185x over previous
"""Trainium2 Bass kernel for nn_AttentionModule (Bahdanau-style attention).

Reference computation (S=512, B=64, H=1024, F=2H):
    cat    = concat([hidden bcast to (S,B,H), encoder_states], -1)      [S,B,2H]
    scores = tanh(cat @ W_attn.T + b_attn) @ W_attn2.T + b_attn2        [S,B,1]
    attn   = softmax(scores[..., 0].T, axis=-1)                         [B,S]
    applied= einsum("bs,sbh->bh", attn, encoder_states)                 [B,H]
    out    = tanh(concat([decoder_out, applied], -1) @ W_comb.T + b_comb)

Sharding: data-parallel over B across 8 cores (8 batch rows per core).

v4 structure:
  - Main matmul T^T[f,s] = sum_h W2T[h,f]*encT[h,s] in fp8e4m3 DoubleRow
    (256 contraction rows per instruction).  W2 host-scaled by 256; the
    tanh undoes it via its fused scale=1/256.  W2T is laid out per-f-tile
    so the first DR group only waits on a 128KB DMA.
  - The first four DR groups of batch row 0 are emitted BEFORE the
    preamble so the PE has work while W1 streams in.
  - hid@W1.T preamble: W1 resident [128, KH*F], 16x256KB DMAs issued on
    the (idle) scalar queue in consumption order; 32 bf16 matmuls + PE
    transposes.  Scores bias hidb folded into tanh as per-partition bias.
  - Scores matmul (attn2) in bf16 with W_attn2 replicated to 128 columns:
    the psum scores tile [128, S] carries the row broadcast for free
    (fp8 tanh outputs measurably hurt the attention weights, so attn2
    stays bf16).
  - Softmax skips max-subtraction (scores are provably tiny); Exp fuses
    the sum via accum_out.
  - applied^T: bf16 mult+reduce per h-chunk on VectorE over a bf16
    encoder copy (gpsimd is too slow per-op to help).
  - enc fp8/bf16 copies streamed per batch row (prefetch one ahead).
  - Final combine bf16; decoder half emitted before the last batch row
    so the PE tail only waits on the last row's applied.
"""

import numpy as np

S, B, H = 512, 64, 1024
F = 2 * H
NCORES = 8
BL = B // NCORES          # 8 batch rows per core
KH = H // 128             # 8 contraction chunks over H
KF = F // 128             # 16 feature tiles
W2SCALE = 256.0           # host pre-scale on W2 for fp8 range

_CACHE = {}


def _build(num_devices=NCORES):
    from contextlib import ExitStack

    import concourse.tile as tile
    from concourse import bacc, mybir
    from concourse.masks import make_identity

    f32 = mybir.dt.float32
    bf16 = mybir.dt.bfloat16
    f8 = mybir.dt.float8e4
    AF = mybir.ActivationFunctionType
    ALU = mybir.AluOpType
    AX = mybir.AxisListType
    DR = mybir.MatmulPerfMode.DoubleRow

    nc = bacc.Bacc("TRN2", target_bir_lowering=False, debug=False,
                   num_devices=num_devices)

    # enc free layout per partition: [b, kc, s]; h = kc*128 + p
    enc8_d = nc.dram_tensor("enc8", [128, BL * KH * S], f8,
                            kind="ExternalInput").ap()
    encb_d = nc.dram_tensor("encb", [128, BL * KH * S], bf16,
                            kind="ExternalInput").ap()
    # w2t8 free layout: [ft, kc, f]; wat1 free layout: [kc, f]
    w2t8_d = nc.dram_tensor("w2t8", [128, KF * KH * 128], f8,
                            kind="ExternalInput").ap()
    wat1_d = nc.dram_tensor("wat1", [128, KH * F], bf16,
                            kind="ExternalInput").ap()
    wct = nc.dram_tensor("wct", [F, H], bf16, kind="ExternalInput").ap()
    hidT = nc.dram_tensor("hidT", [H, BL], f32, kind="ExternalInput").ap()
    decT = nc.dram_tensor("decT", [H, BL], f32, kind="ExternalInput").ap()
    # [128, (ft c)] fp8, value 256*W_attn2[ft*128+p] replicated over c
    w2rep = nc.dram_tensor("w2rep", [128, KF * 128], f8,
                           kind="ExternalInput").ap()
    b_attn_d = nc.dram_tensor("b_attn", [1, F], f32, kind="ExternalInput").ap()
    b_comb_d = nc.dram_tensor("b_comb", [1, H], f32, kind="ExternalInput").ap()
    out_d = nc.dram_tensor("out", [BL, H], f32, kind="ExternalOutput").ap()
    appT_d = nc.dram_tensor("appliedT", [H, BL], f32,
                            kind="ExternalOutput").ap()

    with tile.TileContext(nc) as tc:
        with ExitStack() as ctx:
            consts = ctx.enter_context(tc.tile_pool(name="consts", bufs=1))
            enc8_pool = ctx.enter_context(tc.tile_pool(name="enc8p", bufs=2))
            encb_pool = ctx.enter_context(tc.tile_pool(name="encbp", bufs=2))
            tanh_pool = ctx.enter_context(tc.tile_pool(name="tanh", bufs=18))
            attn_pool = ctx.enter_context(tc.tile_pool(name="attn", bufs=2))
            scr_pool = ctx.enter_context(tc.tile_pool(name="scr", bufs=2))
            small_pool = ctx.enter_context(tc.tile_pool(name="small", bufs=4))
            wct_pool = ctx.enter_context(tc.tile_pool(name="wct", bufs=4))
            psT_pool = ctx.enter_context(
                tc.tile_pool(name="psT", bufs=4, space="PSUM"))
            psSc_pool = ctx.enter_context(
                tc.tile_pool(name="psSc", bufs=2, space="PSUM"))
            psPre_pool = ctx.enter_context(
                tc.tile_pool(name="psPre", bufs=2, space="PSUM"))

            # ---- W1 on the scalar queue (idle early), consumption order ----
            w1_sb = consts.tile([128, KH * F], bf16)
            for j in range(2):
                for kc in range(KH):
                    nc.scalar.dma_start(
                        w1_sb[:, kc * F + j * 1024: kc * F + (j + 1) * 1024],
                        wat1_d[:, kc * F + j * 1024: kc * F + (j + 1) * 1024])

            # ---- sync queue: W2 head chunk, enc b0, then the rest ----
            w2t8_sb = consts.tile([128, KF * KH * 128], f8)
            CW = KH * 128

            def w2_load(ft):
                nc.sync.dma_start(w2t8_sb[:, ft * CW:(ft + 1) * CW],
                                  w2t8_d[:, ft * CW:(ft + 1) * CW])

            def enc8_load(b, nsplit=2):
                t = enc8_pool.tile([128, KH * S], f8, tag="enc8", name="enc8")
                q = KH * S // nsplit
                for i in range(nsplit):
                    nc.sync.dma_start(
                        t[:, i * q:(i + 1) * q],
                        enc8_d[:, b * KH * S + i * q:
                               b * KH * S + (i + 1) * q])
                return t

            def encb_load(b):
                t = encb_pool.tile([128, KH * S], bf16, tag="encb",
                                   name="encb")
                q = KH * S // 4
                for i in range(4):
                    nc.sync.dma_start(
                        t[:, i * q:(i + 1) * q],
                        encb_d[:, b * KH * S + i * q:
                               b * KH * S + (i + 1) * q])
                return t

            w2_load(0)
            enc8_tiles = {0: enc8_load(0, nsplit=4)}
            for ft in range(1, 6):
                w2_load(ft)

            # ---- small constants (shipped fp32, cast on device) ----
            identity = consts.tile([128, 128], f32)
            make_identity(nc, identity[:])
            ones_bf = consts.tile([1, BL], bf16)
            nc.vector.memset(ones_bf[:], 1.0)
            hidT_32 = consts.tile([128, KH * BL], f32)
            for kc in range(KH):
                nc.sync.dma_start(hidT_32[:, kc * BL:(kc + 1) * BL],
                                  hidT[kc * 128:(kc + 1) * 128, :])
            hidT_sb = consts.tile([128, KH * BL], bf16)
            nc.vector.tensor_copy(hidT_sb[:], hidT_32[:])
            b_attn_32 = consts.tile([1, F], f32)
            nc.sync.dma_start(b_attn_32[:], b_attn_d[:])
            b_attn_sb = consts.tile([1, F], bf16)
            nc.vector.tensor_copy(b_attn_sb[:], b_attn_32[:])
            w2rep8_sb = consts.tile([128, KF * 128], f8)
            nc.sync.dma_start(w2rep8_sb[:], w2rep[:])
            w2rep8_v = w2rep8_sb.rearrange("p (t c) -> p t c", t=KF)

            for ft in range(6, KF):
                w2_load(ft)

            decT_32 = consts.tile([128, KH * BL], f32)
            for kc in range(KH):
                nc.sync.dma_start(decT_32[:, kc * BL:(kc + 1) * BL],
                                  decT[kc * 128:(kc + 1) * 128, :])
            decT_sb = consts.tile([128, KH * BL], bf16)
            nc.vector.tensor_copy(decT_sb[:], decT_32[:])
            b_comb_32 = consts.tile([1, H], f32)
            nc.sync.dma_start(b_comb_32[:], b_comb_d[:])
            b_comb_sb = consts.tile([1, H], bf16)
            nc.vector.tensor_copy(b_comb_sb[:], b_comb_32[:])
            encb_tiles = {0: encb_load(0)}

            hidbT_sb = consts.tile([128, KF * BL], f32)
            appT_sb = consts.tile([128, KH * BL], f32)
            appT_bf = consts.tile([128, KH * BL], bf16)
            w2t8_v = w2t8_sb.rearrange("p (t k f) -> p t k f", t=KF, k=KH)

            # ---- prologue: first 6 DR groups of b0 keep the PE busy while
            # W1 streams in for the preamble (the first two borrow the
            # psSc banks, which are idle until b0's scores matmul) ----
            def dr_group(et8_v, ft, pool=None):
                pT = (pool or psT_pool).tile([128, S], f32,
                                             tag="pT" if pool is None
                                             else "psc", name="pT")
                for kc2 in range(KH // 2):
                    nc.tensor.matmul(
                        pT[:],
                        w2t8_v[:, ft, 2 * kc2:2 * kc2 + 2, :],
                        et8_v[:, 2 * kc2:2 * kc2 + 2, :],
                        start=(kc2 == 0), stop=(kc2 == KH // 2 - 1),
                        perf_mode=DR)
                return pT

            et8_b0 = enc8_tiles.pop(0)
            et8_b0_v = et8_b0.rearrange("p (k s) -> p k s", k=KH)
            pT_pending = [dr_group(et8_b0_v, 0, pool=psSc_pool),
                          dr_group(et8_b0_v, 1, pool=psSc_pool)]
            pT_pending += [dr_group(et8_b0_v, ft) for ft in range(2, 6)]

            # ---- hid_part preamble: hidb[b, f] = hidden @ W1.T + b_attn ----
            hidb_row = consts.tile([BL, F], f32)
            for fc in range(F // 512):
                ph = psPre_pool.tile([BL, 512], f32, tag="pre", name=f"ph{fc}")
                for kc in range(KH):
                    nc.tensor.matmul(
                        ph[:], hidT_sb[:, kc * BL:(kc + 1) * BL],
                        w1_sb[:, kc * F + fc * 512: kc * F + (fc + 1) * 512],
                        start=(kc == 0), stop=False)
                nc.tensor.matmul(
                    ph[:], ones_bf[:], b_attn_sb[:, fc * 512:(fc + 1) * 512],
                    start=False, stop=True)
                nc.vector.tensor_copy(hidb_row[:, fc * 512:(fc + 1) * 512],
                                      ph[:])
            for ft in range(KF):
                ptp = psPre_pool.tile([128, BL], f32, tag="pre", name="ptp")
                nc.tensor.transpose(ptp[:],
                                    hidb_row[:, ft * 128:(ft + 1) * 128],
                                    identity[:BL, :BL])
                nc.vector.tensor_copy(hidbT_sb[:, ft * BL:(ft + 1) * BL],
                                      ptp[:])

            # ---- main loop over local batch rows ----
            inv = 1.0 / W2SCALE
            pouts = [None, None]
            for b in range(BL):
                if b + 1 < BL:
                    enc8_tiles[b + 1] = enc8_load(b + 1)
                    encb_tiles[b + 1] = encb_load(b + 1)
                if b > 0:
                    et8 = enc8_tiles.pop(b)
                    et8_v = et8.rearrange("p (k s) -> p k s", k=KH)
                else:
                    et8_v = et8_b0_v

                psc = psSc_pool.tile([128, S], f32, tag="psc", name="psc")
                thp = [None] * (KF // 2)

                for ft in range(KF):
                    if b == 0 and ft < 6:
                        pT = pT_pending[ft]
                    else:
                        pT = dr_group(et8_v, ft)
                    if ft % 2 == 0:
                        thp[ft // 2] = tanh_pool.tile([128, 2 * S], f8,
                                                      tag="tanh", name="tanh")
                    nc.scalar.activation(
                        thp[ft // 2][:, (ft % 2) * S:(ft % 2 + 1) * S],
                        pT[:], AF.Tanh,
                        bias=hidbT_sb[:, ft * BL + b: ft * BL + b + 1],
                        scale=inv)
                # scores matmul as ONE consecutive fp8 DoubleRow group (the
                # lag-interleaved emission of this group miscomputes on HW)
                for fp in range(KF // 2):
                    nc.tensor.matmul(
                        psc[:], w2rep8_v[:, 2 * fp:2 * fp + 2, :],
                        thp[fp].rearrange("p (t s) -> p t s", t=2),
                        start=(fp == 0), stop=(fp == KF // 2 - 1),
                        perf_mode=DR)

                # softmax over s (no max subtraction: |scores| <~ 2)
                attn = attn_pool.tile([128, S], bf16, tag="attn", name="attn")
                sumexp = small_pool.tile([128, 1], f32, tag="sumexp",
                                         name="sumexp")
                nc.scalar.activation(attn[:], psc[:], AF.Exp,
                                     bias=0.0, scale=inv,
                                     accum_out=sumexp[:])
                recip = small_pool.tile([128, 1], f32, tag="recip",
                                        name="recip")
                nc.vector.reciprocal(recip[:], sumexp[:])
                nc.vector.tensor_scalar_mul(attn[:], attn[:], recip[:])

                # applied^T[h, b]: bf16 mult+reduce on VectorE.  The last
                # row's tail gates the final combine, so gpsimd takes three
                # of its multiplies to shorten the critical path.
                etb = encb_tiles.pop(b)
                etb_v = etb.rearrange("p (k s) -> p k s", k=KH)
                gscr = []
                if b == BL - 1:
                    for i, kc in enumerate((5, 6, 7)):
                        g = scr_pool.tile([128, S], bf16, tag=f"gscr{i}",
                                          name="gscr")
                        nc.gpsimd.tensor_tensor(out=g[:], in0=etb_v[:, kc, :],
                                                in1=attn[:], op=ALU.mult)
                        gscr.append(g)
                for kc in range(KH):
                    if b == BL - 1 and kc >= 5:
                        scr = gscr[kc - 5]
                    else:
                        scr = scr_pool.tile([128, S], bf16, tag="scr",
                                            name="scr")
                        nc.vector.tensor_tensor(out=scr[:],
                                                in0=etb_v[:, kc, :],
                                                in1=attn[:], op=ALU.mult)
                    nc.vector.reduce_sum(
                        appT_sb[:, kc * BL + b: kc * BL + b + 1],
                        scr[:], axis=AX.X)

                # decoder half of the final combine before the last row
                if b == BL - 2:
                    for i in range(2):
                        pouts[i] = psPre_pool.tile([BL, 512], f32, tag="pre",
                                                   name=f"po{i}")
                    for kc in range(KH):
                        w = wct_pool.tile([128, H], bf16, tag="wct",
                                          name="wctt")
                        nc.sync.dma_start(w[:], wct[kc * 128:(kc + 1) * 128, :])
                        for fc in range(2):
                            nc.tensor.matmul(
                                pouts[fc][:],
                                decT_sb[:, kc * BL:(kc + 1) * BL],
                                w[:, fc * 512:(fc + 1) * 512],
                                start=(kc == 0), stop=False)

            # ---- final combine: += applied @ Wc_applied.T, bias, tanh ----
            nc.vector.tensor_copy(appT_bf[:], appT_sb[:])
            for kc in range(KH):
                w = wct_pool.tile([128, H], bf16, tag="wct", name="wctt")
                nc.sync.dma_start(w[:], wct[(KH + kc) * 128:
                                            (KH + kc + 1) * 128, :])
                for fc in range(2):
                    nc.tensor.matmul(
                        pouts[fc][:], appT_bf[:, kc * BL:(kc + 1) * BL],
                        w[:, fc * 512:(fc + 1) * 512],
                        start=False, stop=False)
            for fc in range(2):
                nc.tensor.matmul(
                    pouts[fc][:], ones_bf[:],
                    b_comb_sb[:, fc * 512:(fc + 1) * 512],
                    start=False, stop=True)

            out_sb = consts.tile([BL, H], f32)
            for fc in range(2):
                nc.scalar.activation(out_sb[:, fc * 512:(fc + 1) * 512],
                                     pouts[fc][:], AF.Tanh)
            nc.sync.dma_start(out_d[:], out_sb[:])
            for kc in range(KH):
                nc.sync.dma_start(appT_d[kc * 128:(kc + 1) * 128, :],
                                  appT_sb[:, kc * BL:(kc + 1) * BL])

    nc.compile()
    return nc


def _get_nc():
    if "nc" not in _CACHE:
        _CACHE["nc"] = _build()
    return _CACHE["nc"]


def make_in_maps(inputs):
    import ml_dtypes
    bf = ml_dtypes.bfloat16
    f8 = ml_dtypes.float8_e4m3

    inp = {k: np.asarray(v, dtype=np.float32) for k, v in inputs.items()}
    hidden = inp["hidden"]
    decoder_out = inp["decoder_out"]
    encoder_states = inp["encoder_states"]
    W_attn = inp["W_attn"]
    b_attn = inp["b_attn"]
    W_attn2 = inp["W_attn2"]
    W_comb = inp["W_comb"]
    b_comb = inp["b_comb"]
    # b_attn2 shifts every score equally -> softmax-invariant, unused.

    wat = np.ascontiguousarray(W_attn.T)                    # [F, F] fp32
    wat1 = np.ascontiguousarray(
        wat[:H].reshape(KH, 128, F).transpose(1, 0, 2)
        .reshape(128, KH * F)).astype(bf)
    w2t8 = np.ascontiguousarray(
        (wat[H:] * W2SCALE).reshape(KH, 128, KF, 128)
        .transpose(1, 2, 0, 3).reshape(128, KF * KH * 128)).astype(f8)
    wct = np.ascontiguousarray(W_comb.T).astype(bf)
    w2r = (W2SCALE * W_attn2.reshape(KF, 128).T).astype(np.float32)
    w2rep = np.ascontiguousarray(
        np.broadcast_to(w2r[:, :, None], (128, KF, 128))
        .reshape(128, KF * 128)).astype(f8)
    b_attn_2d = np.ascontiguousarray(b_attn.reshape(1, F))
    b_comb_2d = np.ascontiguousarray(b_comb.reshape(1, H))

    in_maps = []
    for c in range(NCORES):
        sl = slice(c * BL, (c + 1) * BL)
        # [S, BL, H] -> [BL, H, S] -> [BL, KH, 128, S] -> [128, BL, KH, S]
        enc = np.ascontiguousarray(
            encoder_states[:, sl, :].transpose(1, 2, 0)
            .reshape(BL, KH, 128, S).transpose(2, 0, 1, 3)
            .reshape(128, BL * KH * S))
        in_maps.append({
            "enc8": enc.astype(f8),
            "encb": enc.astype(bf),
            "w2t8": w2t8,
            "wat1": wat1,
            "wct": wct,
            "hidT": np.ascontiguousarray(hidden[sl].T),
            "decT": np.ascontiguousarray(decoder_out[sl].T),
            "w2rep": w2rep,
            "b_attn": b_attn_2d,
            "b_comb": b_comb_2d,
        })
    return in_maps


def kernel(**inputs):
    from concourse.bass_utils import run_bass_kernel_spmd

    in_maps = make_in_maps(inputs)
    nc = _get_nc()
    res = run_bass_kernel_spmd(nc, in_maps, list(range(NCORES)))
    out = np.concatenate([res.results[c]["out"] for c in range(NCORES)], axis=0)
    applied = np.concatenate(
        [np.ascontiguousarray(res.results[c]["appliedT"].T)
         for c in range(NCORES)], axis=0)
    return out.astype(np.float32), applied.astype(np.float32)
